# revision 9
# baseline (speedup 1.0000x reference)
"""Trainium2 Bass kernel for a 4-layer GCN (PyG GCNConv semantics).

Math: each layer computes  h' = relu(A_hat @ h @ W + b)  where
A_hat = D^-1/2 A D^-1/2 + D^-1 (self loops), D = in-degree + 1.
Aggregation commutes with the dense transform, so each layer aggregates in
whichever of (in_dim, out_dim) is cheaper:
  L1: aggregate x (width 2, host-streamed), then @W1      -> h1 [N,128]
  L2: gather h1 rows (256B bf16), segment-sum, @W2        -> h2 [N,256]
  L3: gather h2 rows (512B bf16), segment-sum, @W3, t=h3@W4 -> t [N,2]
  L4: gather t rows (256B bf16 padded), segment-sum, + b4 -> out [N,2]

Sharding: destination-node slabs. Core c owns 49 blocks x 128 dst nodes.
Edges (+self loops, weights w = dinv[src]*dinv[dst]) are grouped per dst
block, sorted by src, padded to 128-edge tiles (pad idx=-1 so the gather
ucode trims them).  Gather calls cover 4 tiles; each call's int16 indices
are relative to a per-call table base (a 32768-row window that all cores'
edges for that call provably fall in), so no lo/hi phase split is needed.
Calls round-robin over the 4 SWDGE queues.

Per tile: dma_gather 128 table rows -> SBUF [128e, I] bf16; one DVE op
builds a weighted one-hot [128e, 128slots] bf16; TensorE accumulates
aggT[I_chunk, 128slots] += msgs.T @ onehot in PSUM (f32).  Per block: GEMM
(bias via ones-row matmul) + ReLU -> bf16 slab -> AllGather (Shared dst)
-> full table for the next layer's gathers.  Tile counts per block are
max'd across the 8 cores so one NEFF serves all cores (SPMD).
"""

import sys

for _p in ("/opt/trn_rl_repo",):
    if _p not in sys.path:
        sys.path.insert(0, _p)

from contextlib import ExitStack

import numpy as np
import ml_dtypes

import concourse.bass as bass
import concourse.bacc as bacc
import concourse.mybir as mybir
import concourse.tile as tile
from concourse import library_config
from concourse.masks import make_identity

P = 128
NCORES = 8
F32 = mybir.dt.float32
BF16 = mybir.dt.bfloat16
I16 = mybir.dt.int16
I32 = mybir.dt.int32
NPBF16 = ml_dtypes.bfloat16

CALL_T = 4          # tiles per dma_gather call (512 idxs = SWDGE ring cap)
WIN = 32768         # int16-addressable rows per call window
NQUEUES = 1         # SWDGE queues to round-robin gather calls over
NEG_PAD = False     # pad idx=-1 (ucode trims) vs idx=0 (always gathered)


class GCNConfig:
    def __init__(self, n_nodes, dims, blocks_per_core):
        self.n_nodes = n_nodes
        self.dims = list(dims)  # [2, 128, 256, 512, 2]
        self.bpc = blocks_per_core
        self.slab = blocks_per_core * P
        self.npad = NCORES * self.slab
        assert self.npad >= n_nodes
        # padded-row width (gather elem stride must be a multiple of 256B)
        # for the width-2 "t" table: 128 bf16 = 256B
        self.tpad = 128


REAL_CFG = GCNConfig(n_nodes=50000, dims=[2, 128, 256, 512, 2],
                     blocks_per_core=49)


# --------------------------------------------------------------------------
# Host-side graph preprocessing
# --------------------------------------------------------------------------

def preprocess(cfg, edge_index, x):
    """Shard + tile the graph.

    Returns (tiles [bpc] int, calls {j: [(t0, nt, base)]}, per_core dict)."""
    src = np.asarray(edge_index[0], dtype=np.int64)
    dst = np.asarray(edge_index[1], dtype=np.int64)
    n = cfg.n_nodes
    deg = np.bincount(dst, minlength=n).astype(np.float32) + 1.0
    dinv = 1.0 / np.sqrt(deg)

    es = np.concatenate([src, np.arange(n, dtype=np.int64)])
    ed = np.concatenate([dst, np.arange(n, dtype=np.int64)])
    ew = np.concatenate([dinv[src] * dinv[dst], dinv * dinv]).astype(np.float32)

    blk = ed // P
    order = np.lexsort((es, blk))  # by dst block, then src
    es, ed, ew, blk = es[order], ed[order], ew[order], blk[order]

    nblocks = NCORES * cfg.bpc
    cnt = np.bincount(blk, minlength=nblocks)
    cnt_core = cnt.reshape(NCORES, cfg.bpc)
    tiles = (-(-cnt_core // P)).max(axis=0)  # [bpc]
    tiles = np.maximum(tiles, 1)
    tt = int(tiles.sum())

    starts = np.zeros(nblocks + 1, np.int64)
    starts[1:] = np.cumsum(cnt)

    # Per-call table base windows, shared across cores (compile-time).
    # Call (j, ci) covers tiles [4ci, 4ci+nt) of block j on every core; its
    # base is the min src over all cores' edges in those flat positions.
    max_base = cfg.npad - WIN
    calls = []
    for j in range(cfg.bpc):
        T = int(tiles[j])
        ncalls = -(-T // CALL_T)
        cj = []
        for ci in range(ncalls):
            t0 = ci * CALL_T
            nt = min(CALL_T, T - t0)
            lo_s, hi_s = None, None
            for c in range(NCORES):
                g = c * cfg.bpc + j
                s0, s1 = starts[g], starts[g + 1]
                a = s0 + t0 * P
                b = min(s1, s0 + (t0 + nt) * P)
                if a >= b:
                    continue
                mn, mx = int(es[a]), int(es[b - 1])  # sorted by src
                lo_s = mn if lo_s is None else min(lo_s, mn)
                hi_s = mx if hi_s is None else max(hi_s, mx)
            if lo_s is None:
                base = 0
            else:
                base = min(lo_s, max_base)
                assert hi_s - base < WIN, (
                    f"call window overflow: block {j} call {ci}: "
                    f"[{lo_s},{hi_s}] base {base}")
            cj.append((t0, nt, base))
        calls.append(cj)

    xf = np.asarray(x, dtype=np.float32)
    per_core = []
    for c in range(NCORES):
        idx = np.full((tt, P), -1, np.int16)
        dl = np.zeros((tt, P), np.float32)
        wv = np.zeros((tt, P), np.float32)
        xe = np.zeros((tt, P, 2), np.float32)
        cur = 0
        for j in range(cfg.bpc):
            T = int(tiles[j])
            g = c * cfg.bpc + j
            s0, s1 = starts[g], starts[g + 1]
            ne = s1 - s0
            assert ne <= T * P
            flat_i = np.full(T * P, -1, np.int64)
            flat_w = np.zeros(T * P, np.float32)
            flat_d = np.zeros(T * P, np.int64)
            # idx relative to each call's base
            for (t0, nt, base) in calls[j]:
                a, b = t0 * P, min(ne, (t0 + nt) * P)
                if a < b:
                    rel = es[s0 + a:s0 + b] - base
                    assert rel.min() >= 0 and rel.max() < WIN
                    flat_i[a:b] = rel
            flat_w[:ne] = ew[s0:s1]
            flat_d[:ne] = ed[s0:s1] - g * P
            sl = slice(cur, cur + T)
            if NEG_PAD:
                idx[sl] = flat_i.reshape(T, P).astype(np.int16)
            else:
                idx[sl] = np.maximum(flat_i, 0).reshape(T, P).astype(np.int16)
            dl[sl] = flat_d.reshape(T, P).astype(np.float32)
            wv[sl] = flat_w.reshape(T, P)
            xs = np.zeros(T * P, np.int64)
            xs[:ne] = es[s0:s1]
            xe[sl] = xf[xs].reshape(T, P, 2)
            xe[sl][flat_i.reshape(T, P) < 0] = 0.0
            cur += T
        assert cur == tt
        # device idx layout: [128, tt*8] int16 -- per tile, idx i at
        # [i%16 (replicated to all 8 groups), i//16]
        idx_dev = np.zeros((P, tt * 8), np.int16)
        for rep in range(8):
            idx_dev[rep * 16:(rep + 1) * 16] = (
                idx.reshape(tt, 8, 16).transpose(2, 0, 1).reshape(16, tt * 8))
        per_core.append({
            "eidx": idx_dev,
            "edl": dl.T.copy(),                      # [128, tt]
            "ew": wv.T.copy(),                       # [128, tt]
            "exe": xe.transpose(1, 0, 2).reshape(P, tt * 2)
                     .astype(NPBF16).copy(),
        })
    return tiles, calls, per_core


# --------------------------------------------------------------------------
# Device module builder
# --------------------------------------------------------------------------

def build_module(cfg, tiles, calls, debug_layers=4):
    d0, d1, d2, d3, d4 = cfg.dims
    bpc = cfg.bpc
    tt = int(tiles.sum())
    maxt = int(tiles.max())
    nc = bacc.Bacc(None, target_bir_lowering=False, num_swdge_queues=4)

    eidx = nc.declare_dram_parameter("eidx", [P, tt * 8], I16, False)
    edl = nc.declare_dram_parameter("edl", [P, tt], F32, False)
    ew = nc.declare_dram_parameter("ew", [P, tt], F32, False)
    exe = nc.declare_dram_parameter("exe", [P, tt * 2], BF16, False)
    W1 = nc.declare_dram_parameter("W1", [d0, d1], BF16, False)
    W2 = nc.declare_dram_parameter("W2", [d1, d2], BF16, False)
    W3 = nc.declare_dram_parameter("W3", [d2, d3], BF16, False)
    W4 = nc.declare_dram_parameter("W4", [d3, d4], BF16, False)
    b1 = nc.declare_dram_parameter("b1", [1, d1], BF16, False)
    b2 = nc.declare_dram_parameter("b2", [1, d2], BF16, False)
    b3 = nc.declare_dram_parameter("b3", [1, d3], BF16, False)
    b4 = nc.declare_dram_parameter("b4", [1, d4], BF16, False)
    out_p = nc.declare_dram_parameter("out", [cfg.slab, d4], F32, True)

    rg = [list(range(NCORES))]
    eq = mybir.AluOpType.is_equal
    mul = mybir.AluOpType.mult
    relu = mybir.ActivationFunctionType.Relu

    qctr = [0]  # SWDGE queue round-robin

    with tile.TileContext(nc, num_cores=NCORES) as tc, ExitStack() as ctx:
        dram = ctx.enter_context(tc.tile_pool(name="dram", bufs=1, space="DRAM"))
        t1 = dram.tile([cfg.npad, d1], BF16)
        t2 = dram.tile([cfg.npad, d2], BF16)
        tt_tab = dram.tile([cfg.npad, cfg.tpad], BF16)
        ag1 = dram.tile([cfg.slab, d1], BF16)
        ag2 = dram.tile([cfg.slab, d2], BF16)
        agt = dram.tile([cfg.slab, cfg.tpad], BF16)

        const = ctx.enter_context(tc.tile_pool(name="const", bufs=1))
        iota_i = const.tile([P, P], I32)
        iota_f = const.tile([P, P], F32)
        nc.gpsimd.iota(iota_i[:], pattern=[[1, P]], base=0, channel_multiplier=0)
        nc.vector.tensor_copy(iota_f[:], iota_i[:])
        ident = const.tile([P, P], BF16)
        make_identity(nc, ident[:])
        id2 = const.tile([d4, d4], BF16)
        make_identity(nc, id2[:])
        ones = const.tile([1, P], BF16)
        nc.vector.memset(ones[:], 1.0)
        # iota (standard lib) is done; switch GPSIMD ucode to the library
        # that provides DMAGatherAnt
        nc.gpsimd.load_library(library_config.mlp)

        w1s = const.tile([d0, d1], BF16)
        nc.sync.dma_start(out=w1s[:], in_=W1[:, :])
        w2s = const.tile([d1, d2], BF16)
        nc.sync.dma_start(out=w2s[:], in_=W2[:, :])
        w3s = [const.tile([P, d3], BF16, tag=f"w3_{k}", name=f"w3_{k}") for k in range(d2 // P)]
        for k in range(d2 // P):
            nc.sync.dma_start(out=w3s[k][:], in_=W3[k * P:(k + 1) * P, :])
        w4s = [const.tile([P, d4], BF16, tag=f"w4_{k}", name=f"w4_{k}") for k in range(d3 // P)]
        for k in range(d3 // P):
            nc.sync.dma_start(out=w4s[k][:], in_=W4[k * P:(k + 1) * P, :])
        brs = []
        for name, bparam, od in (("b1", b1, d1), ("b2", b2, d2),
                                 ("b3", b3, d3), ("b4", b4, d4)):
            r = const.tile([1, od], BF16, tag=name + "s", name=name + "s")
            nc.sync.dma_start(out=r[:], in_=bparam[:, :])
            brs.append(r)
        b1r, b2r, b3r, b4r = brs

        esb = const.tile([P, tt * 8], I16)
        nc.sync.dma_start(out=esb[:], in_=eidx[:, :])
        dls = const.tile([P, tt], F32)
        nc.sync.dma_start(out=dls[:], in_=edl[:, :])
        ws = const.tile([P, tt], F32)
        nc.sync.dma_start(out=ws[:], in_=ew[:, :])
        xes = const.tile([P, tt * 2], BF16)
        nc.sync.dma_start(out=xes[:], in_=exe[:, :])

        out_acc = const.tile([P, d4 * bpc], F32)
        nc.vector.memset(out_acc[:], 0.0)

        def seg_blocks(lname, table, elem, icols, epilogue, msg_bufs=3):
            """Iterate dst blocks: gather + one-hot seg-matmul, then call
            epilogue(j, aggT_sbuf_chunks)."""
            chunks = [(k * P, min(icols, (k + 1) * P))
                      for k in range(-(-icols // P))]
            with tc.tile_pool(name=lname + "m", bufs=msg_bufs) as mp, \
                 tc.tile_pool(name=lname + "oh", bufs=6) as ohp, \
                 tc.tile_pool(name=lname + "ps", bufs=2, space="PSUM") as pp, \
                 tc.tile_pool(name=lname + "as", bufs=2) as asp:
                if table is not None:
                    # zero-fill the physical msg bufs once: gather calls skip
                    # trailing pad idxs, leaving stale SBUF that must at least
                    # be finite (it is multiplied by w=0).
                    for _ in range(msg_bufs):
                        mz = mp.tile([P, maxt, elem], BF16, tag="msg",
                                     name="mz")
                        nc.vector.memset(mz[:], 0.0)
                cur = 0
                for j in range(bpc):
                    T = int(tiles[j])
                    if table is not None:
                        msg = mp.tile([P, T, elem], BF16, tag="msg")
                        for (t0, nt, base) in calls[j]:
                            hi = min(base + WIN, cfg.npad)
                            nc.gpsimd.dma_gather(
                                out_ap=msg[:, t0:t0 + nt, :],
                                in_ap=table[base:hi, :],
                                idxs_ap=esb[:, (cur + t0) * 8:
                                            (cur + t0 + nt) * 8],
                                num_idxs=nt * P,
                                num_idxs_reg=nt * P,
                                elem_size=elem,
                                queue_num=qctr[0] % NQUEUES,
                            )
                            qctr[0] += 1
                    aggp = [pp.tile([c1 - c0, P], F32, tag=f"agg{k}", name=f"agg{k}")
                            for k, (c0, c1) in enumerate(chunks)]
                    for t in range(T):
                        gt = cur + t
                        oh = ohp.tile([P, P], BF16, tag="oh")
                        nc.vector.scalar_tensor_tensor(
                            out=oh[:], in0=iota_f[:],
                            scalar=dls[:, gt:gt + 1],
                            in1=ws[:, gt:gt + 1].to_broadcast([P, P]),
                            op0=eq, op1=mul)
                        for k, (c0, c1) in enumerate(chunks):
                            if table is not None:
                                lhs = msg[:, t, c0:c1]
                            else:
                                lhs = xes[:, gt * 2:gt * 2 + 2]
                            nc.tensor.matmul(
                                out=aggp[k][:], lhsT=lhs, rhs=oh[:],
                                start=(t == 0), stop=(t == T - 1))
                    aggs = []
                    for k, (c0, c1) in enumerate(chunks):
                        s = asp.tile([c1 - c0, P], BF16, tag=f"aggs{k}", name=f"aggs{k}")
                        nc.scalar.copy(out=s[:], in_=aggp[k][:])
                        aggs.append(s)
                    epilogue(j, aggs)
                    cur += T

        # ---------------- layer 1: x(2) -> h1(d1) -------------------------
        with tc.tile_pool(name="l1e", bufs=2) as ep, \
             tc.tile_pool(name="l1ep", bufs=2, space="PSUM") as epp:
            def epi1(j, aggs):
                hps = epp.tile([P, d1], F32, tag="hp")
                nc.tensor.matmul(out=hps[:], lhsT=aggs[0][:], rhs=w1s[:],
                                 start=True, stop=False)
                nc.tensor.matmul(out=hps[:], lhsT=ones[:], rhs=b1r[:],
                                 start=False, stop=True)
                hsb = ep.tile([P, d1], BF16, tag="h")
                nc.scalar.activation(out=hsb[:], in_=hps[:], func=relu)
                nc.sync.dma_start(out=ag1[j * P:(j + 1) * P, :], in_=hsb[:])
            seg_blocks("l1", None, 0, d0, epi1)
        nc.gpsimd.collective_compute(
            "AllGather", mybir.AluOpType.bypass, replica_groups=rg,
            ins=[ag1[:, :].opt()], outs=[t1[:, :].opt()])

        if debug_layers >= 2:
            # ---------------- layer 2: h1(d1) -> h2(d2) -----------------------
            with tc.tile_pool(name="l2e", bufs=2) as ep, \
                 tc.tile_pool(name="l2ep", bufs=2, space="PSUM") as epp:
                def epi2(j, aggs):
                    hps = epp.tile([P, d2], F32, tag="hp")
                    nc.tensor.matmul(out=hps[:], lhsT=aggs[0][:], rhs=w2s[:],
                                     start=True, stop=False)
                    nc.tensor.matmul(out=hps[:], lhsT=ones[:], rhs=b2r[:],
                                     start=False, stop=True)
                    hsb = ep.tile([P, d2], BF16, tag="h")
                    nc.scalar.activation(out=hsb[:], in_=hps[:], func=relu)
                    nc.sync.dma_start(out=ag2[j * P:(j + 1) * P, :], in_=hsb[:])
                seg_blocks("l2", t1[:, :], d1, d1, epi2)
            nc.gpsimd.collective_compute(
                "AllGather", mybir.AluOpType.bypass, replica_groups=rg,
                ins=[ag2[:, :].opt()], outs=[t2[:, :].opt()])

        if debug_layers >= 3:
            # ------- layer 3: h2(d2) -> h3(d3) -> t = h3@W4 (d4) --------------
            with tc.tile_pool(name="l3e", bufs=2) as ep, \
                 tc.tile_pool(name="l3ep", bufs=1, space="PSUM") as epp:
                def epi3(j, aggs):
                    hps = epp.tile([P, d3], F32, tag="hp")
                    for k in range(d2 // P):
                        nc.tensor.matmul(out=hps[:], lhsT=aggs[k][:], rhs=w3s[k][:],
                                         start=(k == 0), stop=False)
                    nc.tensor.matmul(out=hps[:], lhsT=ones[:], rhs=b3r[:],
                                     start=False, stop=True)
                    hsb = ep.tile([P, d3], BF16, tag="h")
                    nc.scalar.activation(out=hsb[:], in_=hps[:], func=relu)
                    htp = epp.tile([P, d3], BF16, tag="htp")
                    for k in range(d3 // P):
                        nc.tensor.transpose(out=htp[:, k * P:(k + 1) * P],
                                            in_=hsb[:, k * P:(k + 1) * P],
                                            identity=ident[:])
                    hts = ep.tile([P, d3], BF16, tag="hts")
                    nc.scalar.copy(out=hts[:], in_=htp[:])
                    tps = epp.tile([P, d4], F32, tag="tp")
                    for k in range(d3 // P):
                        nc.tensor.matmul(out=tps[:], lhsT=hts[:, k * P:(k + 1) * P],
                                         rhs=w4s[k][:],
                                         start=(k == 0), stop=(k == d3 // P - 1))
                    tsb = ep.tile([P, cfg.tpad], BF16, tag="t")
                    nc.vector.memset(tsb[:], 0.0)
                    nc.scalar.copy(out=tsb[:, 0:d4], in_=tps[:])
                    nc.sync.dma_start(out=agt[j * P:(j + 1) * P, :], in_=tsb[:])
                seg_blocks("l3", t2[:, :], d2, d2, epi3)
            nc.gpsimd.collective_compute(
                "AllGather", mybir.AluOpType.bypass, replica_groups=rg,
                ins=[agt[:, :].opt()], outs=[tt_tab[:, :].opt()])

        if debug_layers >= 4:
            # ---------------- layer 4: t(d4) -> out ---------------------------
            with tc.tile_pool(name="l4ep", bufs=2, space="PSUM") as epp:
                def epi4(j, aggs):
                    ops = epp.tile([P, d4], F32, tag="op")
                    nc.tensor.matmul(out=ops[:], lhsT=aggs[0][:], rhs=id2[:],
                                     start=True, stop=False)
                    nc.tensor.matmul(out=ops[:], lhsT=ones[:], rhs=b4r[:],
                                     start=False, stop=True)
                    nc.scalar.copy(out=out_acc[:, j * d4:(j + 1) * d4], in_=ops[:])
                seg_blocks("l4", tt_tab[:, :], cfg.tpad, d4, epi4)

        nc.sync.dma_start(
            out=out_p[:, :].rearrange("(j p) c -> p j c", p=P),
            in_=out_acc[:].rearrange("p (j c) -> p j c", c=d4))

    return nc


# --------------------------------------------------------------------------
# Entry points
# --------------------------------------------------------------------------

def make_in_maps(cfg, per_core, W1, b1, W2, b2, W3, b3, W4, b4):
    def bf(a):
        return np.ascontiguousarray(np.asarray(a, np.float32).astype(NPBF16))
    shared = {
        "W1": bf(W1), "W2": bf(W2), "W3": bf(W3), "W4": bf(W4),
        "b1": bf(b1).reshape(1, -1), "b2": bf(b2).reshape(1, -1),
        "b3": bf(b3).reshape(1, -1), "b4": bf(b4).reshape(1, -1),
    }
    return [dict(per_core[c], **shared) for c in range(NCORES)]


_CACHE = {}


def _prep_and_build(cfg, x, edge_index):
    tiles, calls, per_core = preprocess(cfg, edge_index, x)
    key = (tuple(tiles.flatten().tolist()),
           tuple((j, t0, nt, base) for j, cj in enumerate(calls)
                 for (t0, nt, base) in cj))
    if key not in _CACHE:
        nc = build_module(cfg, tiles, calls)
        nc.compile()  # Bacc pipeline (reg alloc etc.) before serialization
        _CACHE[key] = nc
    return _CACHE[key], per_core


def _enable_tracing():
    """Make trace=True work in this container: synthesize antenv.axon_hooks
    (the boot image lacks it), register the ctypes NTFF hook, and neuter the
    cloud artifact upload."""
    import types
    import concourse.bass_utils as bu
    try:
        import antenv.axon_hooks  # noqa: F401
    except ImportError:
        import antenv
        mod = types.ModuleType("antenv.axon_hooks")
        holder = {"h": None}
        mod.set_axon_ntff_profile_hook = lambda h: holder.__setitem__("h", h)
        mod.get_axon_ntff_profile_hook = lambda: holder["h"]
        sys.modules["antenv.axon_hooks"] = mod
        antenv.axon_hooks = mod
        if "/root/.axon_site" not in sys.path:
            sys.path.insert(0, "/root/.axon_site")
        from trn_agent_boot.trn_boot import _ntff_profile_via_ctypes
        h = _ntff_profile_via_ctypes("/opt/axon/libaxon_pjrt.so")
        if h is not None:
            mod.set_axon_ntff_profile_hook(h)
    bu.upload_artifacts = lambda tmpdir: tmpdir


def run_on_hw(inputs, trace=False):
    from concourse.bass_utils import run_bass_kernel_spmd
    if trace:
        _enable_tracing()
    cfg = REAL_CFG
    x = np.asarray(inputs["x"], np.float32)
    nc, per_core = _prep_and_build(cfg, x, np.asarray(inputs["edge_index"]))
    in_maps = make_in_maps(cfg, per_core,
                           inputs["W1"], inputs["b1"], inputs["W2"],
                           inputs["b2"], inputs["W3"], inputs["b3"],
                           inputs["W4"], inputs["b4"])
    res = run_bass_kernel_spmd(nc, in_maps, core_ids=list(range(NCORES)),
                               trace=trace)
    out = np.concatenate([res.results[c]["out"] for c in range(NCORES)],
                         axis=0)[:cfg.n_nodes]
    return out.astype(np.float32), res


def kernel(x, edge_index, W1, b1, W2, b2, W3, b3, W4, b4):
    out, _ = run_on_hw(dict(x=x, edge_index=edge_index, W1=W1, b1=b1, W2=W2,
                            b2=b2, W3=W3, b3=b3, W4=W4, b4=b4))
    return out


# revision 10
# speedup vs baseline: 2.8086x; 2.8086x over previous
"""Trainium2 Bass kernel for a 4-layer GCN (PyG GCNConv semantics).

Math: each layer computes  h' = relu(A_hat @ h @ W + b)  where
A_hat = D^-1/2 A D^-1/2 + D^-1 (self loops), D = in-degree + 1.
Aggregation commutes with the dense transform, so each layer aggregates in
whichever of (in_dim, out_dim) is cheaper:
  L1: aggregate x (width 2, host-streamed), then @W1      -> h1 [N,128]
  L2: gather h1 rows (256B bf16), segment-sum, @W2        -> h2 [N,256]
  L3: gather h2 rows (512B bf16), segment-sum, @W3, t=h3@W4 -> t [N,2]
  L4: gather t rows (256B bf16 padded), segment-sum, + b4 -> out [N,2]

Sharding: destination-node slabs. Core c owns 49 blocks x 128 dst nodes.
Edges (+self loops, weights w = dinv[src]*dinv[dst]) are grouped per dst
block, sorted by src, padded to 128-edge tiles (pad idx=-1 so the gather
ucode trims them).  Gather calls cover 4 tiles; each call's int16 indices
are relative to a per-call table base (a 32768-row window that all cores'
edges for that call provably fall in), so no lo/hi phase split is needed.
Calls round-robin over the 4 SWDGE queues.

Per tile: dma_gather 128 table rows -> SBUF [128e, I] bf16; one DVE op
builds a weighted one-hot [128e, 128slots] bf16; TensorE accumulates
aggT[I_chunk, 128slots] += msgs.T @ onehot in PSUM (f32).  Per block: GEMM
(bias via ones-row matmul) + ReLU -> bf16 slab -> AllGather (Shared dst)
-> full table for the next layer's gathers.  Tile counts per block are
max'd across the 8 cores so one NEFF serves all cores (SPMD).
"""

import sys

for _p in ("/opt/trn_rl_repo",):
    if _p not in sys.path:
        sys.path.insert(0, _p)

from contextlib import ExitStack

import numpy as np
import ml_dtypes

import concourse.bass as bass
import concourse.bacc as bacc
import concourse.mybir as mybir
import concourse.tile as tile
from concourse import library_config
from concourse.masks import make_identity

P = 128
NCORES = 8
F32 = mybir.dt.float32
BF16 = mybir.dt.bfloat16
I16 = mybir.dt.int16
I32 = mybir.dt.int32
NPBF16 = ml_dtypes.bfloat16

CALL_T = 4          # tiles per dma_gather call (512 idxs = SWDGE ring cap)
WIN = 32768         # int16-addressable rows per call window
NQUEUES = 4         # SWDGE queues to round-robin gather calls over
NEG_PAD = False     # pad idx=-1 (ucode trims) vs idx=0 (always gathered)


class GCNConfig:
    def __init__(self, n_nodes, dims, blocks_per_core):
        self.n_nodes = n_nodes
        self.dims = list(dims)  # [2, 128, 256, 512, 2]
        self.bpc = blocks_per_core
        self.slab = blocks_per_core * P
        self.npad = NCORES * self.slab
        assert self.npad >= n_nodes
        # padded-row width (gather elem stride must be a multiple of 256B)
        # for the width-2 "t" table: 128 bf16 = 256B
        self.tpad = 128


REAL_CFG = GCNConfig(n_nodes=50000, dims=[2, 128, 256, 512, 2],
                     blocks_per_core=49)


# --------------------------------------------------------------------------
# Host-side graph preprocessing
# --------------------------------------------------------------------------

def preprocess(cfg, edge_index, x):
    """Shard + tile the graph.

    Returns (tiles [bpc] int, calls {j: [(t0, nt, base)]}, per_core dict)."""
    src = np.asarray(edge_index[0], dtype=np.int64)
    dst = np.asarray(edge_index[1], dtype=np.int64)
    n = cfg.n_nodes
    deg = np.bincount(dst, minlength=n).astype(np.float32) + 1.0
    dinv = 1.0 / np.sqrt(deg)

    es = np.concatenate([src, np.arange(n, dtype=np.int64)])
    ed = np.concatenate([dst, np.arange(n, dtype=np.int64)])
    ew = np.concatenate([dinv[src] * dinv[dst], dinv * dinv]).astype(np.float32)

    blk = ed // P
    order = np.lexsort((es, blk))  # by dst block, then src
    es, ed, ew, blk = es[order], ed[order], ew[order], blk[order]

    nblocks = NCORES * cfg.bpc
    cnt = np.bincount(blk, minlength=nblocks)
    cnt_core = cnt.reshape(NCORES, cfg.bpc)
    tiles = (-(-cnt_core // P)).max(axis=0)  # [bpc]
    tiles = np.maximum(tiles, 1)
    tt = int(tiles.sum())

    starts = np.zeros(nblocks + 1, np.int64)
    starts[1:] = np.cumsum(cnt)

    # Per-call table base windows, shared across cores (compile-time).
    # Call (j, ci) covers tiles [4ci, 4ci+nt) of block j on every core; its
    # base is the min src over all cores' edges in those flat positions.
    max_base = cfg.npad - WIN
    calls = []
    for j in range(cfg.bpc):
        T = int(tiles[j])
        ncalls = -(-T // CALL_T)
        cj = []
        for ci in range(ncalls):
            t0 = ci * CALL_T
            nt = min(CALL_T, T - t0)
            lo_s, hi_s = None, None
            for c in range(NCORES):
                g = c * cfg.bpc + j
                s0, s1 = starts[g], starts[g + 1]
                a = s0 + t0 * P
                b = min(s1, s0 + (t0 + nt) * P)
                if a >= b:
                    continue
                mn, mx = int(es[a]), int(es[b - 1])  # sorted by src
                lo_s = mn if lo_s is None else min(lo_s, mn)
                hi_s = mx if hi_s is None else max(hi_s, mx)
            if lo_s is None:
                base = 0
            else:
                base = min(lo_s, max_base)
                assert hi_s - base < WIN, (
                    f"call window overflow: block {j} call {ci}: "
                    f"[{lo_s},{hi_s}] base {base}")
            cj.append((t0, nt, base))
        calls.append(cj)

    xf = np.asarray(x, dtype=np.float32)
    per_core = []
    for c in range(NCORES):
        idx = np.full((tt, P), -1, np.int16)
        dl = np.zeros((tt, P), np.float32)
        wv = np.zeros((tt, P), np.float32)
        xe = np.zeros((tt, P, 2), np.float32)
        cur = 0
        for j in range(cfg.bpc):
            T = int(tiles[j])
            g = c * cfg.bpc + j
            s0, s1 = starts[g], starts[g + 1]
            ne = s1 - s0
            assert ne <= T * P
            flat_i = np.full(T * P, -1, np.int64)
            flat_w = np.zeros(T * P, np.float32)
            flat_d = np.zeros(T * P, np.int64)
            # idx relative to each call's base
            for (t0, nt, base) in calls[j]:
                a, b = t0 * P, min(ne, (t0 + nt) * P)
                if a < b:
                    rel = es[s0 + a:s0 + b] - base
                    assert rel.min() >= 0 and rel.max() < WIN
                    flat_i[a:b] = rel
            flat_w[:ne] = ew[s0:s1]
            flat_d[:ne] = ed[s0:s1] - g * P
            sl = slice(cur, cur + T)
            if NEG_PAD:
                idx[sl] = flat_i.reshape(T, P).astype(np.int16)
            else:
                idx[sl] = np.maximum(flat_i, 0).reshape(T, P).astype(np.int16)
            dl[sl] = flat_d.reshape(T, P).astype(np.float32)
            wv[sl] = flat_w.reshape(T, P)
            xs = np.zeros(T * P, np.int64)
            xs[:ne] = es[s0:s1]
            xe[sl] = xf[xs].reshape(T, P, 2)
            xe[sl][flat_i.reshape(T, P) < 0] = 0.0
            cur += T
        assert cur == tt
        # device idx layout: [128, tt*8] int16 -- per tile, idx i at
        # [i%16 (replicated to all 8 groups), i//16]
        idx_dev = np.zeros((P, tt * 8), np.int16)
        for rep in range(8):
            idx_dev[rep * 16:(rep + 1) * 16] = (
                idx.reshape(tt, 8, 16).transpose(2, 0, 1).reshape(16, tt * 8))
        per_core.append({
            "eidx": idx_dev,
            "edl": dl.T.copy(),                      # [128, tt]
            "ew": wv.T.copy(),                       # [128, tt]
            "exe": xe.transpose(1, 0, 2).reshape(P, tt * 2)
                     .astype(NPBF16).copy(),
        })
    return tiles, calls, per_core


# --------------------------------------------------------------------------
# Device module builder
# --------------------------------------------------------------------------

def build_module(cfg, tiles, calls, debug_layers=4):
    d0, d1, d2, d3, d4 = cfg.dims
    bpc = cfg.bpc
    tt = int(tiles.sum())
    maxt = int(tiles.max())
    nc = bacc.Bacc(None, target_bir_lowering=False, num_swdge_queues=4)

    eidx = nc.declare_dram_parameter("eidx", [P, tt * 8], I16, False)
    edl = nc.declare_dram_parameter("edl", [P, tt], F32, False)
    ew = nc.declare_dram_parameter("ew", [P, tt], F32, False)
    exe = nc.declare_dram_parameter("exe", [P, tt * 2], BF16, False)
    W1 = nc.declare_dram_parameter("W1", [d0, d1], BF16, False)
    W2 = nc.declare_dram_parameter("W2", [d1, d2], BF16, False)
    W3 = nc.declare_dram_parameter("W3", [d2, d3], BF16, False)
    W4 = nc.declare_dram_parameter("W4", [d3, d4], BF16, False)
    b1 = nc.declare_dram_parameter("b1", [1, d1], BF16, False)
    b2 = nc.declare_dram_parameter("b2", [1, d2], BF16, False)
    b3 = nc.declare_dram_parameter("b3", [1, d3], BF16, False)
    b4 = nc.declare_dram_parameter("b4", [1, d4], BF16, False)
    out_p = nc.declare_dram_parameter("out", [cfg.slab, d4], F32, True)

    rg = [list(range(NCORES))]
    eq = mybir.AluOpType.is_equal
    mul = mybir.AluOpType.mult
    relu = mybir.ActivationFunctionType.Relu

    qctr = [0]  # SWDGE queue round-robin

    with tile.TileContext(nc, num_cores=NCORES) as tc, ExitStack() as ctx:
        dram = ctx.enter_context(tc.tile_pool(name="dram", bufs=1, space="DRAM"))
        t1 = dram.tile([cfg.npad, d1], BF16, addr_space="Shared")
        t2 = dram.tile([cfg.npad, d2], BF16, addr_space="Shared")
        tt_tab = dram.tile([cfg.npad, cfg.tpad], BF16, addr_space="Shared")
        ag1 = dram.tile([cfg.slab, d1], BF16)
        ag2 = dram.tile([cfg.slab, d2], BF16)
        agt = dram.tile([cfg.slab, cfg.tpad], BF16)

        const = ctx.enter_context(tc.tile_pool(name="const", bufs=1))
        iota_i = const.tile([P, P], I32)
        iota_f = const.tile([P, P], F32)
        nc.gpsimd.iota(iota_i[:], pattern=[[1, P]], base=0, channel_multiplier=0)
        nc.vector.tensor_copy(iota_f[:], iota_i[:])
        ident = const.tile([P, P], BF16)
        make_identity(nc, ident[:])
        id2 = const.tile([d4, d4], BF16)
        make_identity(nc, id2[:])
        ones = const.tile([1, P], BF16)
        nc.vector.memset(ones[:], 1.0)
        # iota (standard lib) is done; switch GPSIMD ucode to the library
        # that provides DMAGatherAnt
        nc.gpsimd.load_library(library_config.mlp)

        w1s = const.tile([d0, d1], BF16)
        nc.sync.dma_start(out=w1s[:], in_=W1[:, :])
        w2s = const.tile([d1, d2], BF16)
        nc.sync.dma_start(out=w2s[:], in_=W2[:, :])
        w3s = [const.tile([P, d3], BF16, tag=f"w3_{k}", name=f"w3_{k}") for k in range(d2 // P)]
        for k in range(d2 // P):
            nc.sync.dma_start(out=w3s[k][:], in_=W3[k * P:(k + 1) * P, :])
        w4s = [const.tile([P, d4], BF16, tag=f"w4_{k}", name=f"w4_{k}") for k in range(d3 // P)]
        for k in range(d3 // P):
            nc.sync.dma_start(out=w4s[k][:], in_=W4[k * P:(k + 1) * P, :])
        brs = []
        for name, bparam, od in (("b1", b1, d1), ("b2", b2, d2),
                                 ("b3", b3, d3), ("b4", b4, d4)):
            r = const.tile([1, od], BF16, tag=name + "s", name=name + "s")
            nc.sync.dma_start(out=r[:], in_=bparam[:, :])
            brs.append(r)
        b1r, b2r, b3r, b4r = brs

        esb = const.tile([P, tt * 8], I16)
        nc.sync.dma_start(out=esb[:], in_=eidx[:, :])
        dls = const.tile([P, tt], F32)
        nc.sync.dma_start(out=dls[:], in_=edl[:, :])
        ws = const.tile([P, tt], F32)
        nc.sync.dma_start(out=ws[:], in_=ew[:, :])
        xes = const.tile([P, tt * 2], BF16)
        nc.sync.dma_start(out=xes[:], in_=exe[:, :])

        out_acc = const.tile([P, d4 * bpc], F32)
        nc.vector.memset(out_acc[:], 0.0)

        def seg_blocks(lname, table, elem, icols, epilogue, msg_bufs=3):
            """Iterate dst blocks: gather + one-hot seg-matmul, then call
            epilogue(j, aggT_sbuf_chunks)."""
            chunks = [(k * P, min(icols, (k + 1) * P))
                      for k in range(-(-icols // P))]
            with tc.tile_pool(name=lname + "m", bufs=msg_bufs) as mp, \
                 tc.tile_pool(name=lname + "oh", bufs=6) as ohp, \
                 tc.tile_pool(name=lname + "ps", bufs=2, space="PSUM") as pp, \
                 tc.tile_pool(name=lname + "as", bufs=2) as asp:
                if table is not None:
                    # zero-fill the physical msg bufs once: gather calls skip
                    # trailing pad idxs, leaving stale SBUF that must at least
                    # be finite (it is multiplied by w=0).
                    for _ in range(msg_bufs):
                        mz = mp.tile([P, maxt, elem], BF16, tag="msg",
                                     name="mz")
                        nc.vector.memset(mz[:], 0.0)
                cur = 0
                for j in range(bpc):
                    T = int(tiles[j])
                    if table is not None:
                        msg = mp.tile([P, T, elem], BF16, tag="msg")
                        for (t0, nt, base) in calls[j]:
                            hi = min(base + WIN, cfg.npad)
                            nc.gpsimd.dma_gather(
                                out_ap=msg[:, t0:t0 + nt, :],
                                in_ap=table[base:hi, :],
                                idxs_ap=esb[:, (cur + t0) * 8:
                                            (cur + t0 + nt) * 8],
                                num_idxs=nt * P,
                                num_idxs_reg=nt * P,
                                elem_size=elem,
                                queue_num=qctr[0] % NQUEUES,
                            )
                            qctr[0] += 1
                    aggp = [pp.tile([c1 - c0, P], F32, tag=f"agg{k}", name=f"agg{k}")
                            for k, (c0, c1) in enumerate(chunks)]
                    for t in range(T):
                        gt = cur + t
                        oh = ohp.tile([P, P], BF16, tag="oh")
                        nc.vector.scalar_tensor_tensor(
                            out=oh[:], in0=iota_f[:],
                            scalar=dls[:, gt:gt + 1],
                            in1=ws[:, gt:gt + 1].to_broadcast([P, P]),
                            op0=eq, op1=mul)
                        for k, (c0, c1) in enumerate(chunks):
                            if table is not None:
                                lhs = msg[:, t, c0:c1]
                            else:
                                lhs = xes[:, gt * 2:gt * 2 + 2]
                            nc.tensor.matmul(
                                out=aggp[k][:], lhsT=lhs, rhs=oh[:],
                                start=(t == 0), stop=(t == T - 1))
                    aggs = []
                    for k, (c0, c1) in enumerate(chunks):
                        s = asp.tile([c1 - c0, P], BF16, tag=f"aggs{k}", name=f"aggs{k}")
                        nc.scalar.copy(out=s[:], in_=aggp[k][:])
                        aggs.append(s)
                    epilogue(j, aggs)
                    cur += T

        # ---------------- layer 1: x(2) -> h1(d1) -------------------------
        with tc.tile_pool(name="l1e", bufs=2) as ep, \
             tc.tile_pool(name="l1ep", bufs=2, space="PSUM") as epp:
            def epi1(j, aggs):
                hps = epp.tile([P, d1], F32, tag="hp")
                nc.tensor.matmul(out=hps[:], lhsT=aggs[0][:], rhs=w1s[:],
                                 start=True, stop=False)
                nc.tensor.matmul(out=hps[:], lhsT=ones[:], rhs=b1r[:],
                                 start=False, stop=True)
                hsb = ep.tile([P, d1], BF16, tag="h")
                nc.scalar.activation(out=hsb[:], in_=hps[:], func=relu)
                nc.sync.dma_start(out=ag1[j * P:(j + 1) * P, :], in_=hsb[:])
            seg_blocks("l1", None, 0, d0, epi1)
        nc.gpsimd.collective_compute(
            "AllGather", mybir.AluOpType.bypass, replica_groups=rg,
            ins=[ag1[:, :].opt()], outs=[t1[:, :].opt()])

        if debug_layers >= 2:
            # ---------------- layer 2: h1(d1) -> h2(d2) -----------------------
            with tc.tile_pool(name="l2e", bufs=2) as ep, \
                 tc.tile_pool(name="l2ep", bufs=2, space="PSUM") as epp:
                def epi2(j, aggs):
                    hps = epp.tile([P, d2], F32, tag="hp")
                    nc.tensor.matmul(out=hps[:], lhsT=aggs[0][:], rhs=w2s[:],
                                     start=True, stop=False)
                    nc.tensor.matmul(out=hps[:], lhsT=ones[:], rhs=b2r[:],
                                     start=False, stop=True)
                    hsb = ep.tile([P, d2], BF16, tag="h")
                    nc.scalar.activation(out=hsb[:], in_=hps[:], func=relu)
                    nc.sync.dma_start(out=ag2[j * P:(j + 1) * P, :], in_=hsb[:])
                seg_blocks("l2", t1[:, :], d1, d1, epi2)
            nc.gpsimd.collective_compute(
                "AllGather", mybir.AluOpType.bypass, replica_groups=rg,
                ins=[ag2[:, :].opt()], outs=[t2[:, :].opt()])

        if debug_layers >= 3:
            # ------- layer 3: h2(d2) -> h3(d3) -> t = h3@W4 (d4) --------------
            with tc.tile_pool(name="l3e", bufs=2) as ep, \
                 tc.tile_pool(name="l3ep", bufs=1, space="PSUM") as epp:
                def epi3(j, aggs):
                    hps = epp.tile([P, d3], F32, tag="hp")
                    for k in range(d2 // P):
                        nc.tensor.matmul(out=hps[:], lhsT=aggs[k][:], rhs=w3s[k][:],
                                         start=(k == 0), stop=False)
                    nc.tensor.matmul(out=hps[:], lhsT=ones[:], rhs=b3r[:],
                                     start=False, stop=True)
                    hsb = ep.tile([P, d3], BF16, tag="h")
                    nc.scalar.activation(out=hsb[:], in_=hps[:], func=relu)
                    htp = epp.tile([P, d3], BF16, tag="htp")
                    for k in range(d3 // P):
                        nc.tensor.transpose(out=htp[:, k * P:(k + 1) * P],
                                            in_=hsb[:, k * P:(k + 1) * P],
                                            identity=ident[:])
                    hts = ep.tile([P, d3], BF16, tag="hts")
                    nc.scalar.copy(out=hts[:], in_=htp[:])
                    tps = epp.tile([P, d4], F32, tag="tp")
                    for k in range(d3 // P):
                        nc.tensor.matmul(out=tps[:], lhsT=hts[:, k * P:(k + 1) * P],
                                         rhs=w4s[k][:],
                                         start=(k == 0), stop=(k == d3 // P - 1))
                    tsb = ep.tile([P, cfg.tpad], BF16, tag="t")
                    nc.vector.memset(tsb[:], 0.0)
                    nc.scalar.copy(out=tsb[:, 0:d4], in_=tps[:])
                    nc.sync.dma_start(out=agt[j * P:(j + 1) * P, :], in_=tsb[:])
                seg_blocks("l3", t2[:, :], d2, d2, epi3)
            nc.gpsimd.collective_compute(
                "AllGather", mybir.AluOpType.bypass, replica_groups=rg,
                ins=[agt[:, :].opt()], outs=[tt_tab[:, :].opt()])

        if debug_layers >= 4:
            # ---------------- layer 4: t(d4) -> out ---------------------------
            with tc.tile_pool(name="l4ep", bufs=2, space="PSUM") as epp:
                def epi4(j, aggs):
                    ops = epp.tile([P, d4], F32, tag="op")
                    nc.tensor.matmul(out=ops[:], lhsT=aggs[0][:], rhs=id2[:],
                                     start=True, stop=False)
                    nc.tensor.matmul(out=ops[:], lhsT=ones[:], rhs=b4r[:],
                                     start=False, stop=True)
                    nc.scalar.copy(out=out_acc[:, j * d4:(j + 1) * d4], in_=ops[:])
                seg_blocks("l4", tt_tab[:, :], cfg.tpad, d4, epi4)

        nc.sync.dma_start(
            out=out_p[:, :].rearrange("(j p) c -> p j c", p=P),
            in_=out_acc[:].rearrange("p (j c) -> p j c", c=d4))

    return nc


# --------------------------------------------------------------------------
# Entry points
# --------------------------------------------------------------------------

def make_in_maps(cfg, per_core, W1, b1, W2, b2, W3, b3, W4, b4):
    def bf(a):
        return np.ascontiguousarray(np.asarray(a, np.float32).astype(NPBF16))
    shared = {
        "W1": bf(W1), "W2": bf(W2), "W3": bf(W3), "W4": bf(W4),
        "b1": bf(b1).reshape(1, -1), "b2": bf(b2).reshape(1, -1),
        "b3": bf(b3).reshape(1, -1), "b4": bf(b4).reshape(1, -1),
    }
    return [dict(per_core[c], **shared) for c in range(NCORES)]


_CACHE = {}


def _prep_and_build(cfg, x, edge_index):
    tiles, calls, per_core = preprocess(cfg, edge_index, x)
    key = (tuple(tiles.flatten().tolist()),
           tuple((j, t0, nt, base) for j, cj in enumerate(calls)
                 for (t0, nt, base) in cj))
    if key not in _CACHE:
        nc = build_module(cfg, tiles, calls)
        nc.compile()  # Bacc pipeline (reg alloc etc.) before serialization
        _CACHE[key] = nc
    return _CACHE[key], per_core


def _enable_tracing():
    """Make trace=True work in this container: synthesize antenv.axon_hooks
    (the boot image lacks it), register the ctypes NTFF hook, and neuter the
    cloud artifact upload."""
    import types
    import concourse.bass_utils as bu
    try:
        import antenv.axon_hooks  # noqa: F401
    except ImportError:
        import antenv
        mod = types.ModuleType("antenv.axon_hooks")
        holder = {"h": None}
        mod.set_axon_ntff_profile_hook = lambda h: holder.__setitem__("h", h)
        mod.get_axon_ntff_profile_hook = lambda: holder["h"]
        sys.modules["antenv.axon_hooks"] = mod
        antenv.axon_hooks = mod
        if "/root/.axon_site" not in sys.path:
            sys.path.insert(0, "/root/.axon_site")
        from trn_agent_boot.trn_boot import _ntff_profile_via_ctypes
        h = _ntff_profile_via_ctypes("/opt/axon/libaxon_pjrt.so")
        if h is not None:
            mod.set_axon_ntff_profile_hook(h)
    bu.upload_artifacts = lambda tmpdir: tmpdir


def run_on_hw(inputs, trace=False):
    from concourse.bass_utils import run_bass_kernel_spmd
    if trace:
        _enable_tracing()
    cfg = REAL_CFG
    x = np.asarray(inputs["x"], np.float32)
    nc, per_core = _prep_and_build(cfg, x, np.asarray(inputs["edge_index"]))
    in_maps = make_in_maps(cfg, per_core,
                           inputs["W1"], inputs["b1"], inputs["W2"],
                           inputs["b2"], inputs["W3"], inputs["b3"],
                           inputs["W4"], inputs["b4"])
    res = run_bass_kernel_spmd(nc, in_maps, core_ids=list(range(NCORES)),
                               trace=trace)
    out = np.concatenate([res.results[c]["out"] for c in range(NCORES)],
                         axis=0)[:cfg.n_nodes]
    return out.astype(np.float32), res


def kernel(x, edge_index, W1, b1, W2, b2, W3, b3, W4, b4):
    out, _ = run_on_hw(dict(x=x, edge_index=edge_index, W1=W1, b1=b1, W2=W2,
                            b2=b2, W3=W3, b3=b3, W4=W4, b4=b4))
    return out
